# revision 19
# baseline (speedup 1.0000x reference)
"""Trainium2 Bass kernel for nn_ConvPixelToCapsules.

Reference computation:
  x (16, 256, 1, 20, 20) --conv W (256,1,9,9) stride 2--> votes (16,256,32,8,6,6)
  3 dynamic-routing iterations (softmax over co, weighted sum over ci,
  squash over no, agreement update) -> activation (16, 32, 8, 6, 6)

Sharding: data-parallel over batch, 2 batch elements per core on 8 cores.

Per-core design (v2):
  - Host builds an im2col view of x: xim[k=(ky,kx), plane, pos] bf16,
    so the conv is 36 K=81 bf16 matmuls per 128-plane chunk
    (lhsT = im2col slice [81, 128], rhs = W [81, 256]) instead of K=9 fp32.
  - votes stored [plane(128 x4 chunks), (pos, no, co)] bf16.  Putting `no`
    in the middle keeps the innermost AP dim packed for every big DVE
    multiply (2x perf mode) while the squash still reduces over `no` via a
    strided view.
  - route-weighted reduce over ci done with ones-vector matmuls:
    out[pos-pair partition, 512] per batch elem, N=512-wide streams.
  - distances (dot over no) via tree adds on views (packed, 2x mode)
    instead of a strided TENSOR_REDUCE.
  - big elementwise work split between vector and gpsimd engines.
"""

import sys
import functools
import numpy as np

sys.path.insert(0, "/opt/trn_rl_repo")

import concourse.bass as bass  # noqa: E402
import concourse.tile as tile  # noqa: E402
from concourse import mybir  # noqa: E402
from concourse.bass_utils import run_bass_kernel_spmd  # noqa: E402

F32 = mybir.dt.float32
BF16 = mybir.dt.bfloat16

BS, CI, HI, WI = 16, 256, 20, 20
CO, NO, ITERS = 32, 8, 3
KH = KW = 9
K81 = KH * KW               # 81
HOUT = WOUT = 6
POS = HOUT * WOUT           # 36
NCORES = 8
BSH = BS // NCORES          # 2 batch elements per core
PLANES = BSH * CI           # 512
NCHUNK = PLANES // 128      # 4 chunks of 128 planes (b-major)
CONO = CO * NO              # 256
CHW = POS * CONO            # 9216 free elements per chunk of votes (pos, no, co)
HWCO = POS * CO             # 1152 logits free elements per chunk (pos, co)
PP = POS // 2               # 18 pos-pairs
PF = 2 * CONO               # 512 free elements per pos-pair (par, no, co)

Alu = mybir.AluOpType
Act = mybir.ActivationFunctionType
AxX = mybir.AxisListType.X


def ap(t, offset, dims):
    """Explicit AP on the same tensor as `t` (an AP), offset in elements."""
    return bass.AP(tensor=t.tensor, offset=t.offset + offset, ap=[list(d) for d in dims])


def _split_excess_waits(nc):
    """Walrus (enable-ldw-opt=false) allows only ONE sync-wait on DMA and
    Matmult/Ldweights pseudo-structs.  Tile sometimes emits 2+ (WAR + WAW).
    Splice a same-engine NoOp carrying the overflow waits in front."""
    import bass_rust

    nid = 0
    for f in nc.m.functions:
        for blk in f.blocks:
            out = []
            changed = False
            for ins in blk.instructions:
                si = ins.sync_info
                if si is not None and len(si.on_wait) > 1:
                    extra = list(si.on_wait)[:-1]
                    keep = list(si.on_wait)[-1:]
                    for w in extra:
                        nop = bass_rust.InstNoOp(name=f"I-waitnop-{nid}")
                        nid += 1
                        nop.engine = ins.engine
                        nop.sync_info = bass_rust.SyncInfo(on_wait=[w], on_update=[])
                        out.append(nop)
                    ins.sync_info = bass_rust.SyncInfo(
                        on_wait=keep, on_update=list(si.on_update))
                    changed = True
                out.append(ins)
            if changed:
                blk.instructions = out


def build_program(split_waits=True):
    nc = bass.Bass("TRN2", target_bir_lowering=False, debug=False)
    xim_d = nc.dram_tensor("xim", [K81, PLANES * POS], BF16, kind="ExternalInput").ap()
    w_d = nc.dram_tensor("w", [K81, CONO], BF16, kind="ExternalInput").ap()
    b_d = nc.dram_tensor("b", [CONO], F32, kind="ExternalInput").ap()
    # out[b, (pp, par, no, co)] fp32; host transposes to [b, co, no, h, w]
    out_d = nc.dram_tensor("out", [BSH, POS * CONO], F32, kind="ExternalOutput").ap()

    with tile.TileContext(nc) as tc:
        _emit(tc, nc, xim_d, w_d, b_d, out_d)
    if split_waits:
        _split_excess_waits(nc)
    return nc


def _emit(tc, nc, xim_d, w_d, b_d, out_d):
    import contextlib

    with contextlib.ExitStack() as ctx:
        persist = ctx.enter_context(tc.tile_pool(name="persist", bufs=1))
        dram = ctx.enter_context(tc.tile_pool(name="dram", bufs=1, space="DRAM"))
        rt_ps = ctx.enter_context(tc.tile_pool(name="rt_ps", bufs=1, space="PSUM"))

        # ---- persistent tiles ----
        votes = [persist.tile([128, CHW], BF16, name=f"votes{c}") for c in range(NCHUNK)]
        logits = [persist.tile([128, HWCO], F32, name=f"logits{c}") for c in range(NCHUNK)]
        bias_bc = persist.tile([PP, PF], F32, name="bias_bc")
        ones_bf = nc.const_aps.tensor(1.0, (128, 1), BF16)

        # bias: dram [CONO] (no,co) -> sbuf [PP, (par, no, co)]
        nc.sync.dma_start(out=bias_bc[:], in_=ap(b_d, 0, [[0, PP], [0, 2], [1, CONO]]))

        # raw preact accumulators in SBUF [PP, PF] per batch elem (pre-bias)
        pre0 = [persist.tile([PP, PF], F32, name=f"pre0_{b}") for b in range(BSH)]
        stg = ctx.enter_context(tc.tile_pool(name="stg", bufs=1))

        def preact_reduce(srcs, b):
            """pre0[b][t, :] = sum over the 256 planes of batch elem b of
            srcs[2b]/[2b+1][:, t*PF:(t+1)*PF].  One [1,512] PSUM bank per
            pos-pair; engines only address partitions 0/32/64/96, so bounce
            through a partition-0 staging tile and DMA into row t."""
            for t in range(PP):
                ps1 = rt_ps.tile([1, PF], F32, name="ps1", tag=f"ps1_{t % 5}",
                                 bufs=1)
                for k in (0, 1):
                    nc.tensor.matmul(
                        ps1[:], ones_bf,
                        ap(srcs[2 * b + k][:], t * PF, [[CHW, 128], [1, PF]]),
                        start=(k == 0), stop=(k == 1),
                        skip_group_check=True,
                    )
                st = stg.tile([1, PF], F32, name="st", tag=f"st{t % 3}")
                if t % 2 == 0:
                    nc.scalar.copy(out=st[:], in_=ps1[:])
                else:
                    nc.vector.tensor_copy(out=st[:], in_=ps1[:])
                deng = (nc.sync, nc.scalar, nc.gpsimd)[t % 3]
                deng.dma_start(
                    out=ap(pre0[b][:], t * PF, [[PF, 1], [1, PF]]), in_=st[:])

        # ================= CONV =================
        with tc.tile_pool(name="conv_in", bufs=1) as conv_in, \
             tc.tile_pool(name="conv_w", bufs=1) as conv_w, \
             tc.tile_pool(name="conv_ps", bufs=3, space="PSUM") as conv_ps:
            w_sb = conv_w.tile([K81, CONO], BF16, name="w_sb")
            nc.sync.dma_start(out=w_sb[:], in_=ap(w_d, 0, [[CONO, K81], [1, CONO]]))
            for c in range(NCHUNK):
                im = conv_in.tile([K81, 128 * POS], BF16, name="im", tag="im")
                nc.sync.dma_start(
                    out=im[:],
                    in_=ap(xim_d, c * 128 * POS, [[PLANES * POS, K81], [1, 128 * POS]]),
                )
                for p2 in range(PP):
                    cps = conv_ps.tile([128, PF], F32, name="cps", tag="cps")
                    for par in range(2):
                        pos = 2 * p2 + par
                        lhsT = ap(im[:], pos, [[128 * POS, K81], [POS, 128]])
                        nc.tensor.matmul(
                            cps[:, par * CONO:(par + 1) * CONO], lhsT, w_sb[:],
                            start=True, stop=True,
                        )
                    dst = ap(votes[c][:], p2 * PF, [[CHW, 128], [1, PF]])
                    if p2 % 2 == 0:
                        nc.scalar.copy(out=dst, in_=cps[:])
                    else:
                        nc.vector.tensor_copy(out=dst, in_=cps[:])
                # iter-0 preact: uniform route folds into a plain plane-sum
                if c % 2 == 1:
                    preact_reduce(votes, c // 2)

        # ================= ROUTING =================
        small = ctx.enter_context(tc.tile_pool(name="small", bufs=1))
        big = ctx.enter_context(tc.tile_pool(name="big", bufs=3))
        tree = ctx.enter_context(tc.tile_pool(name="tree", bufs=1))

        def big_eng(c):
            return nc.vector if c < 3 else nc.gpsimd

        for t in range(ITERS):
            # --- route & route-weighted votes (t >= 1) + preact ---
            if t > 0:
                red = []
                for c in range(NCHUNK):
                    route = small.tile([128, HWCO], BF16, name="route",
                                       tag=f"route{c % 2}")
                    nc.scalar.activation(out=route[:], in_=logits[c][:],
                                         func=Act.Exp, scale=1.0)
                    z = small.tile([128, POS], F32, name="z", tag=f"z{c % 2}")
                    nc.vector.tensor_reduce(
                        out=z[:], in_=ap(route[:], 0, [[HWCO, 128], [CO, POS], [1, CO]]),
                        axis=AxX, op=Alu.add)
                    rz = small.tile([128, POS], F32, name="rz", tag=f"rz{c % 2}")
                    nc.vector.reciprocal(out=rz[:], in_=z[:])
                    nc.vector.tensor_tensor(
                        route[:], route[:],
                        ap(rz[:], 0, [[POS, 128], [1, POS], [0, CO]]), Alu.mult)
                    mrt = big.tile([128, CHW], BF16, name="mrt", tag="bigring")
                    r_b = ap(route[:], 0, [[HWCO, 128], [CO, POS], [0, NO], [1, CO]])
                    big_eng(c).tensor_tensor(mrt[:], votes[c][:], r_b, Alu.mult)
                    red.append(mrt)
                    if c % 2 == 1:
                        preact_reduce(red, c // 2)

            # --- squash per batch elem ---
            acts = []
            for b in range(BSH):
                preb = small.tile([PP, PF], F32, name="preb", tag=f"preb{b}")
                nc.vector.scalar_tensor_tensor(
                    preb[:], pre0[b][:], (1.0 / CO) if t == 0 else 1.0, bias_bc[:],
                    Alu.mult, Alu.add)
                sq = small.tile([PP, PF], F32, name="sq", tag=f"sq{b}")
                nc.vector.tensor_tensor(sq[:], preb[:], preb[:], Alu.mult)
                s2 = small.tile([PP, 2 * CO], F32, name="s2", tag=f"s2{b}")
                nc.vector.tensor_reduce(
                    out=s2[:],
                    in_=ap(sq[:], 0, [[PF, PP], [CONO, 2], [1, CO], [CO, NO]]),
                    axis=AxX, op=Alu.add)
                nrm = small.tile([PP, 2 * CO], F32, name="nrm", tag=f"nrm{b}")
                nc.scalar.activation(out=nrm[:], in_=s2[:], func=Act.Sqrt, scale=1.0)
                d1 = small.tile([PP, 2 * CO], F32, name="d1", tag=f"d1{b}")
                nc.vector.tensor_scalar_add(d1[:], s2[:], 1.0)
                r1 = small.tile([PP, 2 * CO], F32, name="r1", tag=f"r1{b}")
                nc.vector.reciprocal(out=r1[:], in_=d1[:])
                fac = small.tile([PP, 2 * CO], F32, name="fac", tag=f"fac{b}")
                nc.vector.tensor_tensor(fac[:], nrm[:], r1[:], Alu.mult)
                fac_b = ap(fac[:], 0, [[2 * CO, PP], [CO, 2], [0, NO], [1, CO]])
                if t == ITERS - 1:
                    af = small.tile([PP, PF], F32, name="af", tag=f"af{b}")
                    nc.vector.tensor_tensor(af[:], preb[:], fac_b, Alu.mult)
                    nc.sync.dma_start(
                        out=ap(out_d, b * POS * CONO, [[PF, PP], [1, PF]]),
                        in_=af[:],
                    )
                else:
                    ab = small.tile([PP, PF], BF16, name="ab", tag=f"ab{b}")
                    nc.vector.tensor_tensor(ab[:], preb[:], fac_b, Alu.mult)
                    acts.append(ab)

            if t == ITERS - 1:
                break

            # --- distances: broadcast act, big multiply, tree-reduce over no ---
            abcs = {}
            for c in range(NCHUNK):
                b = c // 2
                if c % 2 == 0:
                    adr = dram.tile([PP, PF], BF16, name="adr", tag=f"adr{b}")
                    deng = nc.sync if b == 0 else nc.scalar
                    deng.dma_start(out=adr[:], in_=acts[b][:])
                    abc = big.tile([128, CHW], BF16, name="abc", tag="bigring")
                    deng.dma_start(out=abc[:], in_=ap(adr[:], 0, [[0, 128], [1, CHW]]))
                    abcs[b] = abc
                eng = big_eng(c)
                md = big.tile([128, CHW], BF16, name="md", tag="bigring")
                eng.tensor_tensor(md[:], votes[c][:], abcs[b][:], Alu.mult)
                tg = "tv" if eng is nc.vector else "tg"
                t4 = tree.tile([128, CHW // 2], BF16, name="t4", tag=tg)
                # fold no 8->4->2 with packed views, then 2->1 into logits
                eng.tensor_tensor(
                    t4[:],
                    ap(md[:], 0, [[CHW, 128], [CONO, POS], [1, 4 * CO]]),
                    ap(md[:], 4 * CO, [[CHW, 128], [CONO, POS], [1, 4 * CO]]),
                    Alu.add)
                eng.tensor_tensor(
                    ap(t4[:], 0, [[CHW // 2, 128], [4 * CO, POS], [1, 2 * CO]]),
                    ap(t4[:], 0, [[CHW // 2, 128], [4 * CO, POS], [1, 2 * CO]]),
                    ap(t4[:], 2 * CO, [[CHW // 2, 128], [4 * CO, POS], [1, 2 * CO]]),
                    Alu.add)
                l_in0 = ap(t4[:], 0, [[CHW // 2, 128], [4 * CO, POS], [1, CO]])
                l_in1 = ap(t4[:], CO, [[CHW // 2, 128], [4 * CO, POS], [1, CO]])
                if t == 0:
                    eng.tensor_tensor(logits[c][:], l_in0, l_in1, Alu.add)
                else:
                    d = small.tile([128, HWCO], BF16, name="d", tag=f"d{c % 2}")
                    eng.tensor_tensor(d[:], l_in0, l_in1, Alu.add)
                    eng.tensor_tensor(logits[c][:], logits[c][:], d[:], Alu.add)


@functools.cache
def _program():
    return build_program()


def _host_inputs(x, W, bias):
    """Build per-core input maps: im2col x (bf16), W columns in (no, co)
    order (bf16), bias flat (no, co) order (f32)."""
    import ml_dtypes

    x = np.asarray(x, dtype=np.float32)
    W = np.asarray(W, dtype=np.float32)
    bias = np.asarray(bias, dtype=np.float32)

    w_t = np.ascontiguousarray(
        W.reshape(CO, NO, K81).transpose(2, 1, 0).reshape(K81, CONO)
    ).astype(ml_dtypes.bfloat16)
    b_flat = np.ascontiguousarray(bias.reshape(CO, NO).T.reshape(CONO))

    in_maps = []
    for i in range(NCORES):
        xs = x[i * BSH:(i + 1) * BSH].reshape(PLANES, HI, WI)
        win = np.lib.stride_tricks.sliding_window_view(xs, (KH, KW), axis=(1, 2))
        win = win[:, ::2, ::2]                    # [plane, oy, ox, ky, kx]
        imcol = win.transpose(3, 4, 0, 1, 2).reshape(K81, PLANES * POS)
        in_maps.append({
            "xim": np.ascontiguousarray(imcol).astype(ml_dtypes.bfloat16),
            "w": w_t,
            "b": b_flat,
        })
    return in_maps


def kernel(x, W, bias, **_ignored):
    nc = _program()
    in_maps = _host_inputs(x, W, bias)
    res = run_bass_kernel_spmd(nc, in_maps, list(range(NCORES)))
    outs = []
    for i in range(NCORES):
        o = res.results[i]["out"].reshape(BSH, PP, 2, NO, CO)
        outs.append(
            np.ascontiguousarray(o.transpose(0, 4, 3, 1, 2))
            .reshape(BSH, CO, NO, HOUT, WOUT))
    return np.ascontiguousarray(np.concatenate(outs, axis=0))


if __name__ == "__main__":
    xs = np.random.randn(BS, CI, 1, HI, WI).astype(np.float32)
    ws = (np.random.randn(CONO, 1, KH, KW) * 0.05).astype(np.float32)
    bs_ = (np.random.randn(CO, NO, 1, 1) * 0.01).astype(np.float32)
    y = kernel(xs, ws, bs_, quantization_bits=8, quantization_bits_routing=8)
    print(y.shape, y.dtype)


# revision 25
# speedup vs baseline: 1.0352x; 1.0352x over previous
"""Trainium2 Bass kernel for nn_ConvPixelToCapsules.

Reference computation:
  x (16, 256, 1, 20, 20) --conv W (256,1,9,9) stride 2--> votes (16,256,32,8,6,6)
  3 dynamic-routing iterations (softmax over co, weighted sum over ci,
  squash over no, agreement update) -> activation (16, 32, 8, 6, 6)

Sharding: data-parallel over batch, 2 batch elements per core on 8 cores.

Per-core design (v2):
  - Host builds an im2col view of x: xim[k=(ky,kx), plane, pos] bf16,
    so the conv is 36 K=81 bf16 matmuls per 128-plane chunk
    (lhsT = im2col slice [81, 128], rhs = W [81, 256]) instead of K=9 fp32.
  - votes stored [plane(128 x4 chunks), (pos, no, co)] bf16.  Putting `no`
    in the middle keeps the innermost AP dim packed for every big DVE
    multiply (2x perf mode) while the squash still reduces over `no` via a
    strided view.
  - route-weighted reduce over ci done with ones-vector matmuls:
    out[pos-pair partition, 512] per batch elem, N=512-wide streams.
  - distances (dot over no) via tree adds on views (packed, 2x mode)
    instead of a strided TENSOR_REDUCE.
  - big elementwise work split between vector and gpsimd engines.
"""

import sys
import functools
import numpy as np

sys.path.insert(0, "/opt/trn_rl_repo")

import concourse.bass as bass  # noqa: E402
import concourse.tile as tile  # noqa: E402
from concourse import mybir  # noqa: E402
from concourse.bass_utils import run_bass_kernel_spmd  # noqa: E402

F32 = mybir.dt.float32
BF16 = mybir.dt.bfloat16

BS, CI, HI, WI = 16, 256, 20, 20
CO, NO, ITERS = 32, 8, 3
KH = KW = 9
K81 = KH * KW               # 81
HOUT = WOUT = 6
POS = HOUT * WOUT           # 36
NCORES = 8
BSH = BS // NCORES          # 2 batch elements per core
PLANES = BSH * CI           # 512
NCHUNK = PLANES // 128      # 4 chunks of 128 planes (b-major)
CONO = CO * NO              # 256
CHW = POS * CONO            # 9216 free elements per chunk of votes (pos, no, co)
HWCO = POS * CO             # 1152 logits free elements per chunk (pos, co)
PP = POS // 2               # 18 pos-pairs
PF = 2 * CONO               # 512 free elements per pos-pair (par, no, co)

Alu = mybir.AluOpType
Act = mybir.ActivationFunctionType
AxX = mybir.AxisListType.X


def ap(t, offset, dims):
    """Explicit AP on the same tensor as `t` (an AP), offset in elements."""
    return bass.AP(tensor=t.tensor, offset=t.offset + offset, ap=[list(d) for d in dims])


def _split_excess_waits(nc):
    """Walrus (enable-ldw-opt=false) allows only ONE sync-wait on DMA and
    Matmult/Ldweights pseudo-structs.  Tile sometimes emits 2+ (WAR + WAW).
    Splice a same-engine NoOp carrying the overflow waits in front."""
    import bass_rust

    nid = 0
    for f in nc.m.functions:
        for blk in f.blocks:
            out = []
            changed = False
            for ins in blk.instructions:
                si = ins.sync_info
                if si is not None and len(si.on_wait) > 1:
                    extra = list(si.on_wait)[:-1]
                    keep = list(si.on_wait)[-1:]
                    for w in extra:
                        nop = bass_rust.InstNoOp(name=f"I-waitnop-{nid}")
                        nid += 1
                        nop.engine = ins.engine
                        nop.sync_info = bass_rust.SyncInfo(on_wait=[w], on_update=[])
                        out.append(nop)
                    ins.sync_info = bass_rust.SyncInfo(
                        on_wait=keep, on_update=list(si.on_update))
                    changed = True
                out.append(ins)
            if changed:
                blk.instructions = out


def build_program(split_waits=True):
    nc = bass.Bass("TRN2", target_bir_lowering=False, debug=False)
    xim_d = nc.dram_tensor("xim", [K81, PLANES * POS], BF16, kind="ExternalInput").ap()
    w_d = nc.dram_tensor("w", [K81, CONO], BF16, kind="ExternalInput").ap()
    b_d = nc.dram_tensor("b", [CONO], F32, kind="ExternalInput").ap()
    # out[b, (pp, par, no, co)] fp32; host transposes to [b, co, no, h, w]
    out_d = nc.dram_tensor("out", [BSH, POS * CONO], F32, kind="ExternalOutput").ap()

    with tile.TileContext(nc) as tc:
        _emit(tc, nc, xim_d, w_d, b_d, out_d)
    if split_waits:
        _split_excess_waits(nc)
    return nc


def _emit(tc, nc, xim_d, w_d, b_d, out_d):
    import contextlib

    with contextlib.ExitStack() as ctx:
        persist = ctx.enter_context(tc.tile_pool(name="persist", bufs=1))
        dram = ctx.enter_context(tc.tile_pool(name="dram", bufs=1, space="DRAM"))
        rt_ps = ctx.enter_context(tc.tile_pool(name="rt_ps", bufs=1, space="PSUM"))

        # ---- persistent tiles ----
        votes = [persist.tile([128, CHW], BF16, name=f"votes{c}") for c in range(NCHUNK)]
        logits = [persist.tile([128, HWCO], F32, name=f"logits{c}") for c in range(NCHUNK)]
        bias_bc = persist.tile([PP, PF], F32, name="bias_bc")
        ones_bf = nc.const_aps.tensor(1.0, (128, 1), BF16)

        # bias: dram [CONO] (no,co) -> sbuf [PP, (par, no, co)]
        nc.sync.dma_start(out=bias_bc[:], in_=ap(b_d, 0, [[0, PP], [0, 2], [1, CONO]]))

        # raw preact accumulators in SBUF [PP, PF] per batch elem (pre-bias)
        pre0 = [persist.tile([PP, PF], F32, name=f"pre0_{b}") for b in range(BSH)]
        stg = ctx.enter_context(tc.tile_pool(name="stg", bufs=1))

        def preact_reduce(srcs, b):
            """pre0[b][t, :] = sum over the 256 planes of batch elem b of
            srcs[2b]/[2b+1][:, t*PF:(t+1)*PF].  One [1,1024] PSUM tile per
            pos-pair PAIR; engines only address partitions 0/32/64/96, so
            bounce through a partition-0 staging tile and DMA into rows."""
            for tq in range(PP // 2):
                ps1 = rt_ps.tile([1, 2 * PF], F32, name="ps1", tag=f"ps1_{tq % 2}",
                                 bufs=1)
                for half in range(2):
                    t = 2 * tq + half
                    for k in (0, 1):
                        nc.tensor.matmul(
                            ps1[:, half * PF:(half + 1) * PF], ones_bf,
                            ap(srcs[2 * b + k][:], t * PF, [[CHW, 128], [1, PF]]),
                            start=(k == 0), stop=(k == 1),
                            skip_group_check=True,
                        )
                st = stg.tile([1, 2 * PF], F32, name="st", tag=f"st{tq % 3}")
                if tq % 2 == 0:
                    nc.scalar.copy(out=st[:], in_=ps1[:])
                else:
                    nc.vector.tensor_copy(out=st[:], in_=ps1[:])
                deng = (nc.sync, nc.scalar, nc.gpsimd)[tq % 3]
                deng.dma_start(
                    out=ap(pre0[b][:], 2 * tq * PF, [[PF, 2], [1, PF]]), in_=st[:])

        # ================= CONV =================
        with tc.tile_pool(name="conv_in", bufs=2) as conv_in, \
             tc.tile_pool(name="conv_w", bufs=1) as conv_w, \
             tc.tile_pool(name="conv_ps", bufs=3, space="PSUM") as conv_ps:
            w_sb = conv_w.tile([K81, CONO], BF16, name="w_sb")
            nc.sync.dma_start(out=w_sb[:], in_=ap(w_d, 0, [[CONO, K81], [1, CONO]]))
            for c in range(NCHUNK):
                im = conv_in.tile([K81, 128 * POS], BF16, name="im", tag="im")
                nc.sync.dma_start(
                    out=im[:],
                    in_=ap(xim_d, c * 128 * POS, [[PLANES * POS, K81], [1, 128 * POS]]),
                )
                for p2 in range(PP):
                    cps = conv_ps.tile([128, PF], F32, name="cps", tag="cps")
                    for par in range(2):
                        pos = 2 * p2 + par
                        lhsT = ap(im[:], pos, [[128 * POS, K81], [POS, 128]])
                        nc.tensor.matmul(
                            cps[:, par * CONO:(par + 1) * CONO], lhsT, w_sb[:],
                            start=True, stop=True,
                        )
                    dst = ap(votes[c][:], p2 * PF, [[CHW, 128], [1, PF]])
                    if p2 % 3 != 1:
                        nc.scalar.copy(out=dst, in_=cps[:])
                    else:
                        nc.vector.tensor_copy(out=dst, in_=cps[:])
                # iter-0 preact: uniform route folds into a plain plane-sum
                if c % 2 == 1:
                    preact_reduce(votes, c // 2)

        # ================= ROUTING =================
        small = ctx.enter_context(tc.tile_pool(name="small", bufs=1))
        big = ctx.enter_context(tc.tile_pool(name="big", bufs=3))
        tree = ctx.enter_context(tc.tile_pool(name="tree", bufs=1))

        def big_eng(c):
            return nc.vector if c < 2 else nc.gpsimd

        for t in range(ITERS):
            # --- route & route-weighted votes (t >= 1) + preact ---
            if t > 0:
                red = []
                for c in range(NCHUNK):
                    route = small.tile([128, HWCO], BF16, name="route",
                                       tag=f"route{c % 2}")
                    nc.scalar.activation(out=route[:], in_=logits[c][:],
                                         func=Act.Exp, scale=1.0)
                    z = small.tile([128, POS], F32, name="z", tag=f"z{c % 2}")
                    nc.vector.tensor_reduce(
                        out=z[:], in_=ap(route[:], 0, [[HWCO, 128], [CO, POS], [1, CO]]),
                        axis=AxX, op=Alu.add)
                    rz = small.tile([128, POS], F32, name="rz", tag=f"rz{c % 2}")
                    nc.vector.reciprocal(out=rz[:], in_=z[:])
                    nc.vector.tensor_tensor(
                        route[:], route[:],
                        ap(rz[:], 0, [[POS, 128], [1, POS], [0, CO]]), Alu.mult)
                    mrt = big.tile([128, CHW], BF16, name="mrt", tag="bigring")
                    r_b = ap(route[:], 0, [[HWCO, 128], [CO, POS], [0, NO], [1, CO]])
                    big_eng(c).tensor_tensor(mrt[:], votes[c][:], r_b, Alu.mult)
                    red.append(mrt)
                    if c % 2 == 1:
                        preact_reduce(red, c // 2)

            # --- squash per batch elem ---
            acts = []
            for b in range(BSH):
                preb = small.tile([PP, PF], F32, name="preb", tag=f"preb{b}")
                nc.vector.scalar_tensor_tensor(
                    preb[:], pre0[b][:], (1.0 / CO) if t == 0 else 1.0, bias_bc[:],
                    Alu.mult, Alu.add)
                sq = small.tile([PP, PF], F32, name="sq", tag=f"sq{b}")
                nc.vector.tensor_tensor(sq[:], preb[:], preb[:], Alu.mult)
                s2 = small.tile([PP, 2 * CO], F32, name="s2", tag=f"s2{b}")
                nc.vector.tensor_reduce(
                    out=s2[:],
                    in_=ap(sq[:], 0, [[PF, PP], [CONO, 2], [1, CO], [CO, NO]]),
                    axis=AxX, op=Alu.add)
                nrm = small.tile([PP, 2 * CO], F32, name="nrm", tag=f"nrm{b}")
                nc.scalar.activation(out=nrm[:], in_=s2[:], func=Act.Sqrt, scale=1.0)
                d1 = small.tile([PP, 2 * CO], F32, name="d1", tag=f"d1{b}")
                nc.vector.tensor_scalar_add(d1[:], s2[:], 1.0)
                r1 = small.tile([PP, 2 * CO], F32, name="r1", tag=f"r1{b}")
                nc.vector.reciprocal(out=r1[:], in_=d1[:])
                fac = small.tile([PP, 2 * CO], F32, name="fac", tag=f"fac{b}")
                nc.vector.tensor_tensor(fac[:], nrm[:], r1[:], Alu.mult)
                fac_b = ap(fac[:], 0, [[2 * CO, PP], [CO, 2], [0, NO], [1, CO]])
                if t == ITERS - 1:
                    af = small.tile([PP, PF], F32, name="af", tag=f"af{b}")
                    nc.vector.tensor_tensor(af[:], preb[:], fac_b, Alu.mult)
                    nc.sync.dma_start(
                        out=ap(out_d, b * POS * CONO, [[PF, PP], [1, PF]]),
                        in_=af[:],
                    )
                else:
                    ab = small.tile([PP, PF], BF16, name="ab", tag=f"ab{b}")
                    nc.vector.tensor_tensor(ab[:], preb[:], fac_b, Alu.mult)
                    adr = dram.tile([PP, PF], BF16, name="adr", tag=f"adr{b}")
                    deng = nc.sync if b == 0 else nc.scalar
                    deng.dma_start(out=adr[:], in_=ab[:])
                    acts.append(adr)

            if t == ITERS - 1:
                break

            # --- distances: broadcast act, big multiply, tree-reduce over no ---
            abcs = {}
            for c in range(NCHUNK):
                b = c // 2
                if c % 2 == 0:
                    deng = nc.sync if b == 0 else nc.scalar
                    abc = big.tile([128, CHW], BF16, name="abc", tag="bigring")
                    deng.dma_start(out=abc[:],
                                   in_=ap(acts[b][:], 0, [[0, 128], [1, CHW]]))
                    abcs[b] = abc
                eng = big_eng(c)
                md = big.tile([128, CHW], BF16, name="md", tag="bigring")
                eng.tensor_tensor(md[:], votes[c][:], abcs[b][:], Alu.mult)
                eng = nc.vector  # trees run 2x on DVE, terribly on gpsimd
                t4 = tree.tile([128, CHW // 2], BF16, name="t4", tag="tv")
                # fold no 8->4->2 with packed views, then 2->1 into logits
                eng.tensor_tensor(
                    t4[:],
                    ap(md[:], 0, [[CHW, 128], [CONO, POS], [1, 4 * CO]]),
                    ap(md[:], 4 * CO, [[CHW, 128], [CONO, POS], [1, 4 * CO]]),
                    Alu.add)
                eng.tensor_tensor(
                    ap(t4[:], 0, [[CHW // 2, 128], [4 * CO, POS], [1, 2 * CO]]),
                    ap(t4[:], 0, [[CHW // 2, 128], [4 * CO, POS], [1, 2 * CO]]),
                    ap(t4[:], 2 * CO, [[CHW // 2, 128], [4 * CO, POS], [1, 2 * CO]]),
                    Alu.add)
                l_in0 = ap(t4[:], 0, [[CHW // 2, 128], [4 * CO, POS], [1, CO]])
                l_in1 = ap(t4[:], CO, [[CHW // 2, 128], [4 * CO, POS], [1, CO]])
                if t == 0:
                    eng.tensor_tensor(logits[c][:], l_in0, l_in1, Alu.add)
                else:
                    d = small.tile([128, HWCO], BF16, name="d", tag=f"d{c % 2}")
                    eng.tensor_tensor(d[:], l_in0, l_in1, Alu.add)
                    eng.tensor_tensor(logits[c][:], logits[c][:], d[:], Alu.add)


@functools.cache
def _program():
    return build_program()


def _host_inputs(x, W, bias):
    """Build per-core input maps: im2col x (bf16), W columns in (no, co)
    order (bf16), bias flat (no, co) order (f32)."""
    import ml_dtypes

    x = np.asarray(x, dtype=np.float32)
    W = np.asarray(W, dtype=np.float32)
    bias = np.asarray(bias, dtype=np.float32)

    w_t = np.ascontiguousarray(
        W.reshape(CO, NO, K81).transpose(2, 1, 0).reshape(K81, CONO)
    ).astype(ml_dtypes.bfloat16)
    b_flat = np.ascontiguousarray(bias.reshape(CO, NO).T.reshape(CONO))

    in_maps = []
    for i in range(NCORES):
        xs = x[i * BSH:(i + 1) * BSH].reshape(PLANES, HI, WI)
        win = np.lib.stride_tricks.sliding_window_view(xs, (KH, KW), axis=(1, 2))
        win = win[:, ::2, ::2]                    # [plane, oy, ox, ky, kx]
        imcol = win.transpose(3, 4, 0, 1, 2).reshape(K81, PLANES * POS)
        in_maps.append({
            "xim": np.ascontiguousarray(imcol).astype(ml_dtypes.bfloat16),
            "w": w_t,
            "b": b_flat,
        })
    return in_maps


def kernel(x, W, bias, **_ignored):
    nc = _program()
    in_maps = _host_inputs(x, W, bias)
    res = run_bass_kernel_spmd(nc, in_maps, list(range(NCORES)))
    outs = []
    for i in range(NCORES):
        o = res.results[i]["out"].reshape(BSH, PP, 2, NO, CO)
        outs.append(
            np.ascontiguousarray(o.transpose(0, 4, 3, 1, 2))
            .reshape(BSH, CO, NO, HOUT, WOUT))
    return np.ascontiguousarray(np.concatenate(outs, axis=0))


if __name__ == "__main__":
    xs = np.random.randn(BS, CI, 1, HI, WI).astype(np.float32)
    ws = (np.random.randn(CONO, 1, KH, KW) * 0.05).astype(np.float32)
    bs_ = (np.random.randn(CO, NO, 1, 1) * 0.01).astype(np.float32)
    y = kernel(xs, ws, bs_, quantization_bits=8, quantization_bits_routing=8)
    print(y.shape, y.dtype)
